# revision 4
# baseline (speedup 1.0000x reference)
"""MixAttention Trainium2 kernel.

Reference computation (B=64, N=384, C=768, H=12, hd=64, Nt=128):
    qkv = x @ W_qkv + b_qkv -> q, k, v per head
    t2t: softmax(q[:, :128] @ k[:, :128].T * 1/8) @ v[:, :128]   (template tokens)
    s2a: softmax(q[:, 128:] @ k.T * 1/8) @ v                     (search tokens)
    out = concat @ W_proj + b_proj

Strategy: pure data-parallel over batch, 8 batches per core on 8 cores.
All contractions need x^T / activations^T layouts; x is transposed once on
the host (free vs. NEFF exec time), attention outputs are transposed on the
PE. Matmuls run as float32r (TF32-like, full PE rate at free-dim >= 256);
softmax numerator/values run bf16 (error ~2.5e-3 worst case on P*V term).

Per-core pipeline, per batch b (N=384 tokens = 3 tiles of 128; token tile 0
is exactly the Nt=128 template block):
  1. DMA xT[c-chunk, 384] slices (host-pretransposed).
  2. qT/kT projection: W_qkv[:, :1536] chunks stationary, xT moving ->
     qkT m-tiles [128=2 heads x 64, 384] (+bias via ACT, f32r).
  3. v projection: xT chunks stationary, W_v moving -> token-major v
     [128, 12 heads x 65] with a ones column per head (for softmax sums).
  4. Scores transposed: kT stationary (2 heads packed in row groups 0/64),
     qT moving -> S^T[keys, queries] per head per key-chunk; chunk 0 spans
     all 384 queries (template+search), chunks 1-2 span search queries only.
  5. exp via ACT (scale=1/8 fused) -> E^T bf16.
  6. PV: E^T[keys,128q] stationary, [v_h|1] moving -> psum [queries, 65]
     accumulated over key chunks; col 64 = softmax denominator.
  7. normalize: reciprocal(col64) on DVE, ACT copy cols 0:63 * recip ->
     token-major attention output X [queries, 768].
  8. PE-transpose X -> X^T, then output projection: X^T chunks stationary,
     W_proj moving -> out[tokens, 768] + bias, DMA to DRAM.
"""

import numpy as np

B, N, C = 64, 384, 768
H, HD = 12, 64
NT = 128          # template tokens (t_h * t_w * 2)
NCORES = 8
NB = B // NCORES  # batches per core
TOK = NB * N      # tokens per core

_PROGRAM = None


def _build_program(nbatch, e_bf16=True):
    import concourse.mybir as mybir
    import concourse.tile as tile
    from concourse import bacc

    f32 = mybir.dt.float32
    f32r = mybir.dt.float32r
    bf16 = mybir.dt.bfloat16
    e_dt = bf16 if e_bf16 else f32r
    Act = mybir.ActivationFunctionType

    tok = nbatch * N
    nc = bacc.Bacc("TRN2", target_bir_lowering=False)

    xT = nc.dram_tensor("xT", [C, tok], f32, kind="ExternalInput")
    wqkv = nc.dram_tensor("wqkv", [C, 3 * C], f32, kind="ExternalInput")
    bqkv = nc.dram_tensor("bqkv", [3 * C], f32, kind="ExternalInput")
    wproj = nc.dram_tensor("wproj", [C, C], f32, kind="ExternalInput")
    bproj = nc.dram_tensor("bproj", [C], f32, kind="ExternalInput")
    idd = nc.dram_tensor("idd", [128, 128], f32, kind="ExternalInput")
    out = nc.dram_tensor("out", [tok, C], f32, kind="ExternalOutput")

    NCH = C // 128  # 6 c-chunks

    with tile.TileContext(nc) as tc:
        with (
            tc.tile_pool(name="wpool", bufs=1) as wpool,
            tc.tile_pool(name="xpool", bufs=2) as xpool,
            tc.tile_pool(name="qkpool", bufs=1) as qkpool,
            tc.tile_pool(name="epool", bufs=3) as epool,
            tc.tile_pool(name="vpool", bufs=2) as vpool,
            tc.tile_pool(name="xapool", bufs=1) as xapool,
            tc.tile_pool(name="xt2pool", bufs=1) as xt2pool,
            tc.tile_pool(name="opool", bufs=3) as opool,
            tc.tile_pool(name="rpool", bufs=8) as rpool,
            tc.tile_pool(name="pspool", bufs=5, space="PSUM") as pspool,
            tc.tile_pool(name="pvpool", bufs=2, space="PSUM") as pvpool,
        ):
            # ---- resident weights / constants ----
            w_qk = []
            w_v = []
            w_p = []
            for ci in range(NCH):
                t = wpool.tile([128, 2 * C], f32r, tag=f"wqk{ci}")
                nc.sync.dma_start(t[:], wqkv[ci * 128:(ci + 1) * 128, 0:2 * C]
                                  .bitcast(f32r))
                w_qk.append(t)
                t = wpool.tile([128, C], f32r, tag=f"wv{ci}")
                nc.sync.dma_start(t[:], wqkv[ci * 128:(ci + 1) * 128, 2 * C:3 * C]
                                  .bitcast(f32r))
                w_v.append(t)
                t = wpool.tile([128, C], f32r, tag=f"wp{ci}")
                nc.sync.dma_start(t[:], wproj[ci * 128:(ci + 1) * 128, :]
                                  .bitcast(f32r))
                w_p.append(t)

            ident = wpool.tile([128, 128], f32r, tag="ident")
            nc.sync.dma_start(ident[:], idd[:].bitcast(f32r))

            # q/k bias: [128 partitions, 12 m-tiles]
            bqk = wpool.tile([128, 2 * C // 128], f32, tag="bqk")
            nc.sync.dma_start(
                bqk[:], bqkv[0:2 * C].rearrange("(m p) -> p m", p=128))
            # v / proj biases broadcast to all partitions
            bv_row = wpool.tile([1, C], f32, tag="bvrow")
            nc.sync.dma_start(bv_row[:], bqkv[2 * C:3 * C].rearrange("(a c) -> a c", a=1))
            bv = wpool.tile([128, C], f32, tag="bv")
            nc.gpsimd.partition_broadcast(bv[:], bv_row[:])
            bp_row = wpool.tile([1, C], f32, tag="bprow")
            nc.sync.dma_start(bp_row[:], bproj[:].rearrange("(a c) -> a c", a=1))
            bp = wpool.tile([128, C], f32, tag="bp")
            nc.gpsimd.partition_broadcast(bp[:], bp_row[:])

            for b in range(nbatch):
                # ---- 1. load xT slices ----
                xt = []
                for ci in range(NCH):
                    t = xpool.tile([128, N], f32r, tag=f"xt{ci}")
                    nc.sync.dma_start(
                        t[:],
                        xT[ci * 128:(ci + 1) * 128, b * N:(b + 1) * N]
                        .bitcast(f32r))
                    xt.append(t)

                # ---- 2. q/k projection (transposed outputs) ----
                qk = []
                for mt in range(2 * C // 128):  # 12 m-tiles: q 0-5, k 6-11
                    ps = pspool.tile([128, N], f32, tag="ps")
                    for ci in range(NCH):
                        nc.tensor.matmul(
                            ps[:], w_qk[ci][:, mt * 128:(mt + 1) * 128],
                            xt[ci][:], start=(ci == 0), stop=(ci == NCH - 1))
                    t = qkpool.tile([128, N], f32r, tag=f"qk{mt}")
                    nc.scalar.activation(t[:], ps[:], Act.Identity,
                                         bias=bqk[:, mt:mt + 1], scale=1.0)
                    qk.append(t)

                # ---- 3. v projection (token-major, ones interleaved) ----
                v1 = []
                for tt in range(3):
                    t = vpool.tile([128, H, HD + 1], e_dt, tag=f"v1{tt}")
                    nc.vector.memset(t[:, :, HD:HD + 1], 1.0)
                    for half in range(2):
                        ps = pspool.tile([128, N], f32, tag="ps")
                        for ci in range(NCH):
                            nc.tensor.matmul(
                                ps[:], xt[ci][:, tt * 128:(tt + 1) * 128],
                                w_v[ci][:, half * N:(half + 1) * N],
                                start=(ci == 0), stop=(ci == NCH - 1))
                        nc.vector.scalar_tensor_tensor(
                            out=t[:, 6 * half:6 * half + 6, 0:HD],
                            in0=ps[:].rearrange("p (h d) -> p h d", d=HD),
                            scalar=1.0,
                            in1=bv[:, half * N:(half + 1) * N]
                            .rearrange("p (h d) -> p h d", d=HD),
                            op0=mybir.AluOpType.mult,
                            op1=mybir.AluOpType.add)
                    v1.append(t)

                # ---- 4-7. attention per head ----
                xa = [xapool.tile([128, C], f32r, tag=f"xa{qt}",
                                  name=f"xa{qt}_{b}")
                      for qt in range(3)]
                for hp in range(H // 2):
                    for part in range(2):
                        h = 2 * hp + part
                        base = 64 * part
                        kt = qk[6 + hp]
                        qt_t = qk[hp]
                        # scores^T + exp, per key chunk
                        e_tiles = []
                        for jc in range(3):
                            n0 = 0 if jc == 0 else 128
                            w = N - n0
                            ps = pspool.tile([128, N], f32, tag="ps")
                            nc.tensor.matmul(
                                ps[:, 0:w],
                                kt[base:base + 64, jc * 128:(jc + 1) * 128],
                                qt_t[base:base + 64, n0:N],
                                start=True, stop=True)
                            et = epool.tile([128, N], e_dt, tag=f"e{jc}")
                            nc.scalar.activation(et[:, 0:w], ps[:, 0:w],
                                                 Act.Exp, bias=0.0, scale=0.125)
                            e_tiles.append(et)
                        # PV per query tile (0 = template, 1-2 = search)
                        for qt in range(3):
                            pv = pvpool.tile([128, HD + 1], f32, tag="pspv")
                            if qt == 0:
                                chunks = [(0, 0)]
                            else:
                                chunks = [(0, (qt - 1) * 128 + 128),
                                          (1, (qt - 1) * 128),
                                          (2, (qt - 1) * 128)]
                            for i, (jc, col) in enumerate(chunks):
                                nc.tensor.matmul(
                                    pv[:], e_tiles[jc][:, col:col + 128],
                                    v1[jc][:, h, :],
                                    start=(i == 0), stop=(i == len(chunks) - 1))
                            rc = rpool.tile([128, 1], f32, tag="rc")
                            nc.vector.reciprocal(rc[:], pv[:, HD:HD + 1])
                            nc.scalar.activation(
                                xa[qt][:, h * HD:(h + 1) * HD], pv[:, 0:HD],
                                Act.Copy, bias=0.0, scale=rc[:])

                # ---- 8. transpose X and output projection ----
                xt2 = []
                for ci in range(NCH):
                    t = xt2pool.tile([128, N], f32r, tag=f"xt2{ci}")
                    xt2.append(t)
                for qt in range(3):
                    for ci in range(NCH):
                        ps = pspool.tile([128, 128], f32r, tag="pst", bufs=1)
                        nc.tensor.transpose(
                            ps[:], xa[qt][:, ci * 128:(ci + 1) * 128], ident[:])
                        nc.vector.tensor_copy(
                            xt2[ci][:, qt * 128:(qt + 1) * 128], ps[:])
                for tt in range(3):
                    for half in range(2):
                        ps = pspool.tile([128, N], f32, tag="ps")
                        for ci in range(NCH):
                            nc.tensor.matmul(
                                ps[:], xt2[ci][:, tt * 128:(tt + 1) * 128],
                                w_p[ci][:, half * N:(half + 1) * N],
                                start=(ci == 0), stop=(ci == NCH - 1))
                        ot = opool.tile([128, N], f32, tag="osb")
                        nc.vector.scalar_tensor_tensor(
                            out=ot[:], in0=ps[:], scalar=1.0,
                            in1=bp[:, half * N:(half + 1) * N],
                            op0=mybir.AluOpType.mult, op1=mybir.AluOpType.add)
                        nc.sync.dma_start(
                            out[(b * 3 + tt) * 128:(b * 3 + tt + 1) * 128,
                                half * N:(half + 1) * N], ot[:])
    nc.compile()
    return nc


def _get_program():
    global _PROGRAM
    if _PROGRAM is None:
        _PROGRAM = _build_program(NB)
    return _PROGRAM


def kernel(x, W_qkv, b_qkv, W_proj, b_proj, t_h, t_w, s_h, s_w):
    from concourse.bass_utils import run_bass_kernel_spmd

    x = np.asarray(x, dtype=np.float32)
    W_qkv = np.asarray(W_qkv, dtype=np.float32)
    b_qkv = np.asarray(b_qkv, dtype=np.float32)
    W_proj = np.asarray(W_proj, dtype=np.float32)
    b_proj = np.asarray(b_proj, dtype=np.float32)
    assert x.shape == (B, N, C)
    assert int(t_h) * int(t_w) * 2 == NT
    assert int(s_h) * int(s_w) == N - NT

    nc = _get_program()
    ident = np.eye(128, dtype=np.float32)
    in_maps = []
    for i in range(NCORES):
        xc = x[i * NB:(i + 1) * NB].reshape(TOK, C)
        in_maps.append({
            "xT": np.ascontiguousarray(xc.T),
            "wqkv": W_qkv, "bqkv": b_qkv,
            "wproj": W_proj, "bproj": b_proj,
            "idd": ident,
        })
    res = run_bass_kernel_spmd(nc, in_maps, core_ids=list(range(NCORES)))
    return np.concatenate(
        [r["out"].reshape(NB, N, C) for r in res.results], axis=0)


# revision 6
# speedup vs baseline: 30.9085x; 30.9085x over previous
"""MixAttention Trainium2 kernel.

Reference computation (B=64, N=384, C=768, H=12, hd=64, Nt=128):
    qkv = x @ W_qkv + b_qkv -> q, k, v per head
    t2t: softmax(q[:, :128] @ k[:, :128].T * 1/8) @ v[:, :128]   (template tokens)
    s2a: softmax(q[:, 128:] @ k.T * 1/8) @ v                     (search tokens)
    out = concat @ W_proj + b_proj

Strategy: pure data-parallel over batch, 8 batches per core on 8 cores.
All contractions need x^T / activations^T layouts; x is transposed once on
the host (free vs. NEFF exec time), attention outputs are transposed on the
PE. Matmuls run as float32r (TF32-like, full PE rate at free-dim >= 256);
softmax numerator/values run bf16 (error ~2.5e-3 worst case on P*V term).

Per-core pipeline, per batch b (N=384 tokens = 3 tiles of 128; token tile 0
is exactly the Nt=128 template block):
  1. DMA xT[c-chunk, 384] slices (host-pretransposed).
  2. qT/kT projection: W_qkv[:, :1536] chunks stationary, xT moving ->
     qkT m-tiles [128=2 heads x 64, 384] (+bias via ACT, f32r).
  3. v projection: xT chunks stationary, W_v moving -> token-major v
     [128, 12 heads x 65] with a ones column per head (for softmax sums).
  4. Scores transposed: kT stationary (2 heads packed in row groups 0/64),
     qT moving -> S^T[keys, queries] per head per key-chunk; chunk 0 spans
     all 384 queries (template+search), chunks 1-2 span search queries only.
  5. exp via ACT (scale=1/8 fused) -> E^T bf16.
  6. PV: E^T[keys,128q] stationary, [v_h|1] moving -> psum [queries, 65]
     accumulated over key chunks; col 64 = softmax denominator.
  7. normalize: reciprocal(col64) on DVE, ACT copy cols 0:63 * recip ->
     token-major attention output X [queries, 768].
  8. PE-transpose X -> X^T, then output projection: X^T chunks stationary,
     W_proj moving -> out[tokens, 768] + bias, DMA to DRAM.
"""

import numpy as np

B, N, C = 64, 384, 768
H, HD = 12, 64
NT = 128          # template tokens (t_h * t_w * 2)
NCORES = 8
NB = B // NCORES  # batches per core
TOK = NB * N      # tokens per core

_PROGRAM = None


def _build_program(nbatch, e_bf16=True, loop_reps=1):
    import contextlib
    import concourse.mybir as mybir
    import concourse.tile as tile
    from concourse import bacc

    f32 = mybir.dt.float32
    f32r = mybir.dt.float32r
    bf16 = mybir.dt.bfloat16
    e_dt = bf16 if e_bf16 else f32r
    Act = mybir.ActivationFunctionType

    tok = nbatch * N
    nc = bacc.Bacc("TRN2", target_bir_lowering=False)

    xT = nc.dram_tensor("xT", [C, tok], f32, kind="ExternalInput")
    wqkv = nc.dram_tensor("wqkv", [C, 3 * C], f32, kind="ExternalInput")
    bqkv = nc.dram_tensor("bqkv", [3 * C], f32, kind="ExternalInput")
    wproj = nc.dram_tensor("wproj", [C, C], f32, kind="ExternalInput")
    bproj = nc.dram_tensor("bproj", [C], f32, kind="ExternalInput")
    idd = nc.dram_tensor("idd", [128, 128], f32, kind="ExternalInput")
    out = nc.dram_tensor("out", [tok, C], f32, kind="ExternalOutput")

    NCH = C // 128  # 6 c-chunks

    with tile.TileContext(nc) as tc:
        with (
            tc.tile_pool(name="wpool", bufs=1) as wpool,
            tc.tile_pool(name="xpool", bufs=2) as xpool,
            tc.tile_pool(name="qkpool", bufs=1) as qkpool,
            tc.tile_pool(name="epool", bufs=3) as epool,
            tc.tile_pool(name="vpool", bufs=2) as vpool,
            tc.tile_pool(name="xapool", bufs=1) as xapool,
            tc.tile_pool(name="xt2pool", bufs=1) as xt2pool,
            tc.tile_pool(name="opool", bufs=3) as opool,
            tc.tile_pool(name="rpool", bufs=8) as rpool,
            tc.tile_pool(name="pspool", bufs=5, space="PSUM") as pspool,
            tc.tile_pool(name="pvpool", bufs=2, space="PSUM") as pvpool,
        ):
            # ---- resident weights / constants ----
            w_qk = []
            w_v = []
            w_p = []
            for ci in range(NCH):
                t = wpool.tile([128, 2 * C], f32r, tag=f"wqk{ci}")
                nc.sync.dma_start(t[:], wqkv[ci * 128:(ci + 1) * 128, 0:2 * C]
                                  .bitcast(f32r))
                w_qk.append(t)
                t = wpool.tile([128, C], f32r, tag=f"wv{ci}")
                nc.sync.dma_start(t[:], wqkv[ci * 128:(ci + 1) * 128, 2 * C:3 * C]
                                  .bitcast(f32r))
                w_v.append(t)
                t = wpool.tile([128, C], f32r, tag=f"wp{ci}")
                nc.sync.dma_start(t[:], wproj[ci * 128:(ci + 1) * 128, :]
                                  .bitcast(f32r))
                w_p.append(t)

            ident = wpool.tile([128, 128], f32r, tag="ident")
            nc.sync.dma_start(ident[:], idd[:].bitcast(f32r))

            # q/k bias: [128 partitions, 12 m-tiles]
            bqk = wpool.tile([128, 2 * C // 128], f32, tag="bqk")
            nc.sync.dma_start(
                bqk[:], bqkv[0:2 * C].rearrange("(m p) -> p m", p=128))
            # v / proj biases broadcast to all partitions
            bv_row = wpool.tile([1, C], f32, tag="bvrow")
            nc.sync.dma_start(bv_row[:], bqkv[2 * C:3 * C].rearrange("(a c) -> a c", a=1))
            bv = wpool.tile([128, C], f32, tag="bv")
            nc.gpsimd.partition_broadcast(bv[:], bv_row[:])
            bp_row = wpool.tile([1, C], f32, tag="bprow")
            nc.sync.dma_start(bp_row[:], bproj[:].rearrange("(a c) -> a c", a=1))
            bp = wpool.tile([128, C], f32, tag="bp")
            nc.gpsimd.partition_broadcast(bp[:], bp_row[:])

            loop_cm = (tc.For_i(0, loop_reps, 1) if loop_reps > 1
                       else contextlib.nullcontext())
            with loop_cm:
              for b in range(nbatch):
                # ---- 1. load xT slices ----
                xt = []
                for ci in range(NCH):
                    t = xpool.tile([128, N], f32r, tag=f"xt{ci}")
                    nc.sync.dma_start(
                        t[:],
                        xT[ci * 128:(ci + 1) * 128, b * N:(b + 1) * N]
                        .bitcast(f32r))
                    xt.append(t)

                # ---- 2. q/k projection (transposed outputs) ----
                qk = []
                for mt in range(2 * C // 128):  # 12 m-tiles: q 0-5, k 6-11
                    ps = pspool.tile([128, N], f32, tag="ps")
                    for ci in range(NCH):
                        nc.tensor.matmul(
                            ps[:], w_qk[ci][:, mt * 128:(mt + 1) * 128],
                            xt[ci][:], start=(ci == 0), stop=(ci == NCH - 1))
                    t = qkpool.tile([128, N], f32r, tag=f"qk{mt}")
                    nc.scalar.activation(t[:], ps[:], Act.Identity,
                                         bias=bqk[:, mt:mt + 1], scale=1.0)
                    qk.append(t)

                # ---- 3. v projection (token-major, ones interleaved) ----
                v1 = []
                for tt in range(3):
                    t = vpool.tile([128, H, HD + 1], e_dt, tag=f"v1{tt}")
                    nc.vector.memset(t[:, :, HD:HD + 1], 1.0)
                    for half in range(2):
                        ps = pspool.tile([128, N], f32, tag="ps")
                        for ci in range(NCH):
                            nc.tensor.matmul(
                                ps[:], xt[ci][:, tt * 128:(tt + 1) * 128],
                                w_v[ci][:, half * N:(half + 1) * N],
                                start=(ci == 0), stop=(ci == NCH - 1))
                        nc.vector.scalar_tensor_tensor(
                            out=t[:, 6 * half:6 * half + 6, 0:HD],
                            in0=ps[:].rearrange("p (h d) -> p h d", d=HD),
                            scalar=1.0,
                            in1=bv[:, half * N:(half + 1) * N]
                            .rearrange("p (h d) -> p h d", d=HD),
                            op0=mybir.AluOpType.mult,
                            op1=mybir.AluOpType.add)
                    v1.append(t)

                # ---- 4-7. attention per head ----
                xa = [xapool.tile([128, C], f32r, tag=f"xa{qt}",
                                  name=f"xa{qt}_{b}")
                      for qt in range(3)]
                for hp in range(H // 2):
                    for part in range(2):
                        h = 2 * hp + part
                        base = 64 * part
                        kt = qk[6 + hp]
                        qt_t = qk[hp]
                        # scores^T + exp, per key chunk
                        e_tiles = []
                        for jc in range(3):
                            n0 = 0 if jc == 0 else 128
                            w = N - n0
                            ps = pspool.tile([128, N], f32, tag="ps")
                            nc.tensor.matmul(
                                ps[:, 0:w],
                                kt[base:base + 64, jc * 128:(jc + 1) * 128],
                                qt_t[base:base + 64, n0:N],
                                start=True, stop=True)
                            et = epool.tile([128, N], e_dt, tag=f"e{jc}")
                            nc.scalar.activation(et[:, 0:w], ps[:, 0:w],
                                                 Act.Exp, bias=0.0, scale=0.125)
                            e_tiles.append(et)
                        # PV per query tile (0 = template, 1-2 = search)
                        for qt in range(3):
                            pv = pvpool.tile([128, HD + 1], f32, tag="pspv")
                            if qt == 0:
                                chunks = [(0, 0)]
                            else:
                                chunks = [(0, (qt - 1) * 128 + 128),
                                          (1, (qt - 1) * 128),
                                          (2, (qt - 1) * 128)]
                            for i, (jc, col) in enumerate(chunks):
                                nc.tensor.matmul(
                                    pv[:], e_tiles[jc][:, col:col + 128],
                                    v1[jc][:, h, :],
                                    start=(i == 0), stop=(i == len(chunks) - 1))
                            rc = rpool.tile([128, 1], f32, tag="rc")
                            nc.vector.reciprocal(rc[:], pv[:, HD:HD + 1])
                            nc.scalar.activation(
                                xa[qt][:, h * HD:(h + 1) * HD], pv[:, 0:HD],
                                Act.Copy, bias=0.0, scale=rc[:])

                # ---- 8. transpose X and output projection ----
                xt2 = []
                for ci in range(NCH):
                    t = xt2pool.tile([128, N], f32r, tag=f"xt2{ci}")
                    xt2.append(t)
                for qt in range(3):
                    for ci in range(NCH):
                        ps = pspool.tile([128, 128], f32r, tag="pst", bufs=1)
                        nc.tensor.transpose(
                            ps[:], xa[qt][:, ci * 128:(ci + 1) * 128], ident[:])
                        nc.vector.tensor_copy(
                            xt2[ci][:, qt * 128:(qt + 1) * 128], ps[:])
                for tt in range(3):
                    for half in range(2):
                        ps = pspool.tile([128, N], f32, tag="ps")
                        for ci in range(NCH):
                            nc.tensor.matmul(
                                ps[:], xt2[ci][:, tt * 128:(tt + 1) * 128],
                                w_p[ci][:, half * N:(half + 1) * N],
                                start=(ci == 0), stop=(ci == NCH - 1))
                        ot = opool.tile([128, N], f32, tag="osb")
                        nc.vector.scalar_tensor_tensor(
                            out=ot[:], in0=ps[:], scalar=1.0,
                            in1=bp[:, half * N:(half + 1) * N],
                            op0=mybir.AluOpType.mult, op1=mybir.AluOpType.add)
                        nc.sync.dma_start(
                            out[(b * 3 + tt) * 128:(b * 3 + tt + 1) * 128,
                                half * N:(half + 1) * N], ot[:])
    nc.compile()
    return nc


def _get_program():
    global _PROGRAM
    if _PROGRAM is None:
        _PROGRAM = _build_program(NB)
    return _PROGRAM


def kernel(x, W_qkv, b_qkv, W_proj, b_proj, t_h, t_w, s_h, s_w):
    from concourse.bass_utils import run_bass_kernel_spmd

    x = np.asarray(x, dtype=np.float32)
    W_qkv = np.asarray(W_qkv, dtype=np.float32)
    b_qkv = np.asarray(b_qkv, dtype=np.float32)
    W_proj = np.asarray(W_proj, dtype=np.float32)
    b_proj = np.asarray(b_proj, dtype=np.float32)
    assert x.shape == (B, N, C)
    assert int(t_h) * int(t_w) * 2 == NT
    assert int(s_h) * int(s_w) == N - NT

    nc = _get_program()
    ident = np.eye(128, dtype=np.float32)
    in_maps = []
    for i in range(NCORES):
        xc = x[i * NB:(i + 1) * NB].reshape(TOK, C)
        in_maps.append({
            "xT": np.ascontiguousarray(xc.T),
            "wqkv": W_qkv, "bqkv": b_qkv,
            "wproj": W_proj, "bproj": b_proj,
            "idd": ident,
        })
    res = run_bass_kernel_spmd(nc, in_maps, core_ids=list(range(NCORES)))
    return np.concatenate(
        [r["out"].reshape(NB, N, C) for r in res.results], axis=0)
